# revision 11
# baseline (speedup 1.0000x reference)
"""Trainium2 Bass kernel for nn_Architecture_11879879540882 (AKT-style
monotonic sparse attention), data-parallel over batch on 8 NeuronCores.

Self-contained: hardcodes shapes B=16,S=512,D=256,H=8,DK=32, shards the batch
2-per-core, runs one Bass graph SPMD via run_bass_kernel_spmd, gathers output.

Algorithm notes (validated vs the jax reference in numpy, rel err ~1.4e-3 in
full bf16):
 - blocks 1/2: k-projection shares q weights and inputs -> K == Q.
 - masked softmax + cumsum distance statistics collapse into ONE reversed
   masked scan: state=(E+state)*mask -> all suffix sums + masked row total.
 - dist = sqrt(suffix*pos*g^2/r), total = exp(-dist)   [g = -softplus(gamma)]
 - second softmax is UNMASKED (reproduces the reference's non-inplace
   masked_fill bug); the 1e-5 clip on `total` is skipped (~1e-4 error).
 - all LayerNorm gamma/beta fold into downstream weights on host.
 - block3's query is position-independent -> its score rows are broadcasts.

Hardware constraints honored: matmul partition bases must be in {0,32,64}
(per-head tensors packed along the free axis); SBUF budget ~192KB/partition
(half-block phase granularity + aggressive tile-tag slot reuse).
"""
import sys
import numpy as np

for _p in ('/opt/trn_rl_repo',):
    if _p not in sys.path:
        sys.path.append(_p)

import ml_dtypes
import concourse.bass as bass
import concourse.bacc as bacc
import concourse.tile as tile
import concourse.mybir as mybir
from concourse.bass_utils import run_bass_kernel_spmd

F32 = mybir.dt.float32
BF16 = mybir.dt.bfloat16
Alu = mybir.AluOpType
Act = mybir.ActivationFunctionType
NPBF = ml_dtypes.bfloat16

B, S, D, H, DK = 16, 512, 256, 8, 32
NCORES = 8
BL = B // NCORES          # local batches per core = 2
PC = BL * 4               # 128-row position chunks per core = 8
LN_EPS = 1e-5

REV = (slice(None), slice(None, None, -1))


def _softplus(x):
    return np.logaddexp(0.0, x)


def _host_prep(inp):
    """Parameter preprocessing on host. Returns (consts dict, g2 dict)."""
    p = {k: np.asarray(v, np.float32) for k, v in inp.items()}
    c = {}
    s4 = np.float32(DK ** -0.25)
    bf = lambda x: np.ascontiguousarray(np.asarray(x, np.float32)).astype(NPBF)
    colpack = lambda b: np.ascontiguousarray(
        np.asarray(b, np.float32).reshape(2, 128).T).astype(np.float32)

    for blk in ('b1', 'b2'):
        c[blk + '_wq'] = bf(p[blk + '_qw'] * s4)
        c[blk + '_qbr'] = bf((p[blk + '_qb'] * s4)[None, :])
        c[blk + '_wv'] = bf(p[blk + '_vw'])
        c[blk + '_vbr'] = bf(p[blk + '_vb'][None, :])
        c[blk + '_wo'] = bf(p[blk + '_ow'])
        c[blk + '_obr'] = bf(p[blk + '_ob'][None, :])
    know = p['know'][0, 0]
    q03 = ((know @ p['b3_qw'] + p['b3_qb']) / np.sqrt(DK)).reshape(H, DK)
    Q03 = np.zeros((D, H), np.float32)
    for h in range(H):
        Q03[h * DK:(h + 1) * DK, h] = q03[h]
    c['q03'] = bf(Q03)
    g1, be1 = p['b1_lng'], p['b1_lnb']
    c['b3_wk'] = bf(p['b3_kw'] * g1[:, None])
    c['b3_kbT'] = colpack(p['b3_kb'] + be1 @ p['b3_kw'])
    g2_, be2 = p['b2_lng'], p['b2_lnb']
    c['b3_wv'] = bf(p['b3_vw'] * g2_[:, None])
    c['b3_vbr'] = bf((p['b3_vb'] + be2 @ p['b3_vw'])[None, :])
    c['b3_wo'] = bf(p['b3_ow'])
    c['b3_obr'] = bf((p['b3_ob'] + know)[None, :])
    g3, be3 = p['b3_lng'], p['b3_lnb']
    lvw = np.zeros((H, DK, D), np.float32)
    lvb = np.zeros((H, D), np.float32)
    for h in range(H):
        sl = slice(h * DK, (h + 1) * DK)
        lvw[h] = p['lv_w'] * g3[sl][:, None]
        lvb[h] = p['lv_b'] + be3[sl] @ p['lv_w']
    c['lvw'] = bf(lvw)                            # -> lvw__h [32,256]
    c['lvbr'] = bf(lvb.reshape(1, H * D))         # [1, 2048]
    know_r = know.reshape(H, DK)
    keyh = 1.0 / (1.0 + np.exp(-(know_r @ p['lk_w'] + p['lk_b'])))
    c['keyhT'] = bf(keyh.T)                       # [D, H]

    # padded inclusive mask: mpad[ic][p, j] = (j <= i_p), j in [0, 512];
    # strict mask is the shifted view mpad[:, 1:513].
    i = np.arange(S + 1, dtype=np.int64)
    mpad = np.zeros((4, 128, S + 1), np.float32)
    pos = np.zeros((4, 128, S), np.float32)
    for ic in range(4):
        ii = np.arange(ic * 128, (ic + 1) * 128, dtype=np.int64)[:, None]
        mpad[ic] = (i[None, :] <= ii)
        pos[ic] = np.abs(ii - i[None, :S])
    c['mpad'] = bf(mpad)
    c['posm'] = bf(pos)
    c['ident'] = bf(np.eye(128))

    flat = {}
    for name, a in c.items():
        if a.ndim == 2 and a.shape[0] > 128:
            for kc in range(a.shape[0] // 128):
                flat[f"{name}__{kc}"] = np.ascontiguousarray(
                    a[kc * 128:(kc + 1) * 128])
        elif a.ndim == 3:
            for kc in range(a.shape[0]):
                flat[f"{name}__{kc}"] = np.ascontiguousarray(a[kc])
        else:
            flat[name] = a
    g2 = {blk: [float(v) for v in
                (_softplus(p[blk + '_gam'][:, 0, 0]) ** 2)]
          for blk in ('b1', 'b2', 'b3')}
    return flat, g2


_NPDT = {np.dtype(np.float32): F32, np.dtype(NPBF): BF16}


def _build(consts, g2):
    """Builds the per-core Bass graph (BL local batches)."""
    nc = bacc.Bacc("TRN2", target_bir_lowering=False, debug=False)

    x1d = nc.dram_tensor("x1", [BL, S, D], F32, kind="ExternalInput")
    x2d = nc.dram_tensor("x2", [BL, S, D], F32, kind="ExternalInput")
    outd = nc.dram_tensor("out", [BL, S, D], F32, kind="ExternalOutput")
    cd = {name: nc.dram_tensor(name, list(a.shape), _NPDT[a.dtype],
                               kind="ExternalInput")
          for name, a in consts.items()}

    from contextlib import ExitStack
    with tile.TileContext(nc) as tc, ExitStack() as _ps:
        sb = _ps.enter_context(tc.tile_pool(name="const", bufs=1))
        work = _ps.enter_context(tc.tile_pool(name="work", bufs=1))
        rot = _ps.enter_context(tc.tile_pool(name="rot", bufs=3))
        sm = _ps.enter_context(tc.tile_pool(name="sm", bufs=3))
        p1 = _ps.enter_context(tc.tile_pool(name="p1", bufs=2, space="PSUM"))
        pT = _ps.enter_context(tc.tile_pool(name="pT", bufs=1, space="PSUM"))
        pAV = _ps.enter_context(tc.tile_pool(name="pAV", bufs=2, space="PSUM"))
        pO = _ps.enter_context(tc.tile_pool(name="pO", bufs=1, space="PSUM"))

        # ---------- constants ----------
        C = {}
        for name, ap in cd.items():
            t = sb.tile(list(ap.shape), ap.dtype, name="c_" + name)
            nc.sync.dma_start(t[:], ap[:])
            C[name] = t
        ones1 = sb.tile([1, 512], BF16, name="ones1")
        nc.vector.memset(ones1[:], 1.0)
        epsT = sb.tile([128, 1], F32, name="epsT")
        nc.vector.memset(epsT[:], LN_EPS)
        ident = C['ident']

        def transpose128(dst, src):
            """dst[128,128] SBUF bf16 = src.T via PE + DVE copy."""
            pt = pT.tile([128, 512], BF16, tag="pT", name="pt_t")
            nc.tensor.transpose(pt[:, 0:128], src, ident[:])
            nc.vector.tensor_copy(dst, pt[:, 0:128])

        # ---------- input prep: transposed bf16 copies of x1/x2 ----------
        xT = {}
        for xi, xd in ((1, x1d), (2, x2d)):
            for dc in range(2):
                xT[(xi, dc)] = work.tile([128, BL * S], BF16, tag="xfrm",
                                         bufs=6, name=f"xT{xi}_{dc}")
            for pc in range(PC):
                b, ic = divmod(pc, 4)
                t = sm.tile([128, D], F32, tag="xin", name="xin", bufs=2)
                nc.sync.dma_start(t[:], xd[b, ic * 128:(ic + 1) * 128, :])
                tb = sm.tile([128, D], BF16, tag="xbf", name="xbf_t", bufs=2)
                nc.vector.tensor_copy(tb[:], t[:])
                for dc in range(2):
                    transpose128(xT[(xi, dc)][:, pc * 128:(pc + 1) * 128],
                                 tb[:, dc * 128:(dc + 1) * 128])

        # ---------- projections ----------
        def projQ(xTloc, wname, brname, out_name):
            """Head-packed transposed projection QTp [32, H*BL*S]:
            head h occupies cols [h*1024, (h+1)*1024)."""
            QTp = work.tile([32, H * BL * S], BF16, name=out_name,
                            tag="QTp", bufs=1)
            qbr = C[brname]
            for h in range(H):
                for half in range(BL):
                    ps = p1.tile([32, 512], F32, tag="p1", name="projQ_ps")
                    for kc in range(2):
                        nc.tensor.matmul(
                            ps[:],
                            C[f"{wname}__{kc}"][:, h * 32:(h + 1) * 32],
                            xTloc[kc][:, half * 512:(half + 1) * 512],
                            start=(kc == 0), stop=False)
                    nc.tensor.matmul(ps[:], qbr[0:1, h * 32:(h + 1) * 32],
                                     ones1[0:1, :], start=False, stop=True)
                    nc.vector.tensor_copy(
                        QTp[:, h * 1024 + half * 512:
                            h * 1024 + (half + 1) * 512], ps[:])
            return QTp

        def projT(xTloc, wname, bTname, out_name):
            """Chunk-transposed projection out[dc][128, BL*S] (for K3T)."""
            out = [work.tile([128, BL * S], BF16, tag="xfrm", bufs=6,
                             name=f"{out_name}_{dc}") for dc in range(2)]
            bT = C[bTname]
            for dc in range(2):
                for hh in range(BL):
                    ps = p1.tile([128, 1024], F32, tag="p1", name="projT_ps")
                    for kc in range(2):
                        nc.tensor.matmul(
                            ps[:, 0:512],
                            C[f"{wname}__{kc}"][:, dc * 128:(dc + 1) * 128],
                            xTloc[kc][:, hh * 512:(hh + 1) * 512],
                            start=(kc == 0), stop=(kc == 1))
                    nc.scalar.activation(out[dc][:, hh * 512:(hh + 1) * 512],
                                         ps[:, 0:512], Act.Identity,
                                         bias=bT[:, dc:dc + 1], scale=1.0)
            return out

        def projN(xTloc, wname, brname, out_name):
            """Natural projection out[pc][128,256] = x W + b."""
            out = [work.tile([128, D], BF16, tag="Vt", bufs=8,
                             name=f"{out_name}_{pc}") for pc in range(PC)]
            br = C[brname]
            for pc in range(PC):
                ps = pO.tile([128, D], F32, tag="pO", name="projN_ps")
                for kc in range(2):
                    nc.tensor.matmul(ps[:],
                                     xTloc[kc][:, pc * 128:(pc + 1) * 128],
                                     C[f"{wname}__{kc}"],
                                     start=(kc == 0), stop=False)
                nc.tensor.matmul(ps[:], ones1[0:1, 0:128], br[:],
                                 start=False, stop=True)
                nc.scalar.activation(out[pc][:], ps[:], Act.Copy)
            return out

        # ---------- one attention block (half-block phase granularity) ----
        def emit_block(blk, QTp, V, S3=None, resid_dram=None,
                       out_name="hout"):
            g2l = g2[blk]
            strict = (blk == 'b3')
            mpad = [C[f"mpad__{ic}"] for ic in range(4)]
            masks = [mpad[ic][:, 1:513] if strict else mpad[ic][:, 0:512]
                     for ic in range(4)]
            posm = [C[f"posm__{ic}"] for ic in range(4)]

            def qk_matmul(ps_out, b, h, ic):
                base = h * 1024 + b * 512
                nc.tensor.matmul(
                    ps_out,
                    QTp[:, base + ic * 128: base + ic * 128 + 128],
                    QTp[:, base: base + 512],
                    start=True, stop=True)

            houts = [work.tile([128, D], BF16, tag="hblk", bufs=8,
                               name=f"{out_name}_{pc}") for pc in range(PC)]
            ybufs = {}
            mvb = sm.tile([128, 2 * PC], F32, tag="mvb", name="mvb", bufs=2)
            Wo = [C[blk + '_wo__0'], C[blk + '_wo__1']]
            obr = C[blk + '_obr']

            for hblk in range(2):           # half-block: ics {0,1} / {2,3}
                ics = (2 * hblk, 2 * hblk + 1)
                bigA = {}
                for b in range(BL):
                    for ic in ics:
                        bigA[b * 4 + ic] = work.tile(
                            [128, 8 * 512], BF16, tag="bigA", bufs=4,
                            name="bigA_t")

                # ---- phase 1 ----
                def phase1_tail(Eview, b, h, ic):
                    Rv = sm.tile([128, 513], BF16, tag="Rv", name="Rv")
                    nc.vector.memset(Rv[:, 512:513], 0.0)
                    nc.vector.tensor_tensor_scan(
                        Rv[:, 0:512][REV], Eview[REV], masks[ic][REV],
                        0.0, op0=Alu.add, op1=Alu.mult)
                    r1 = sm.tile([128, 1], F32, tag="r1", name="r1")
                    nc.vector.tensor_scalar(r1[:], Rv[:, 0:1], 1e-30,
                                            None, op0=Alu.max)
                    ir = sm.tile([128, 1], F32, tag="ir", name="ir")
                    nc.vector.reciprocal(ir[:], r1[:])
                    rg = sm.tile([128, 1], F32, tag="rg", name="rg")
                    nc.vector.tensor_scalar(rg[:], ir[:], float(g2l[h]),
                                            None, op0=Alu.mult)
                    nc.vector.scalar_tensor_tensor(
                        bigA[b * 4 + ic][:, h * 512:(h + 1) * 512],
                        Rv[:, 1:513], rg[:], posm[ic][:],
                        op0=Alu.mult, op1=Alu.mult)

                if not strict:
                    for ic in ics:
                        for h in range(H):
                            ps = p1.tile([128, 1024], F32, tag="p1",
                                         name="qk_ps")
                            for b in range(BL):
                                qk_matmul(ps[:, b * 512:(b + 1) * 512],
                                          b, h, ic)
                            Epair = rot.tile([128, 1024], BF16, tag="Epair",
                                             name="Epair")
                            nc.scalar.activation(Epair[:], ps[:], Act.Exp)
                            for b in range(BL):
                                phase1_tail(Epair[:, b * 512:(b + 1) * 512],
                                            b, h, ic)
                else:
                    for h in range(H):
                        for b in range(BL):
                            Eb3 = rot.tile([128, 512], BF16, tag="Eb3",
                                           name="Eb3")
                            nc.scalar.activation(
                                Eb3[:], S3[h][:, b * 512:(b + 1) * 512],
                                Act.Exp)
                            for ic in ics:
                                phase1_tail(Eb3[:], b, h, ic)

                # ---- phase 2: total = exp(-sqrt(v)) ----
                # (all Sqrt first, then all Exp: each Sqrt<->Exp transition
                # costs a ~2.7us ACT table-set reload)
                dbigs = {}
                for g in sorted(bigA):
                    dbig = rot.tile([128, 8 * 512], BF16, tag="dbig",
                                    name="dbig", bufs=4)
                    nc.scalar.activation(dbig[:], bigA[g][:], Act.Sqrt)
                    dbigs[g] = dbig
                for g in sorted(bigA):
                    nc.scalar.activation(bigA[g][:], dbigs[g][:], Act.Exp,
                                         scale=-1.0)

                # ---- phase 3 ----
                for b in range(BL):
                    for ic in ics:
                        g = b * 4 + ic
                        pc = g
                        pav = pAV.tile([128, 256], F32, tag="pAV",
                                       name="pav")
                        for h in range(H):
                            tslice = bigA[g][:, h * 512:(h + 1) * 512]
                            if not strict:
                                ps3 = p1.tile([128, 1024], F32, tag="p1",
                                              name="s3_ps")
                                qk_matmul(ps3[:, 0:512], b, h, ic)
                                sview = ps3[:, 0:512]
                            else:
                                sview = S3[h][:, b * 512:(b + 1) * 512]
                            z = sm.tile([128, 512], BF16, tag="z", name="z")
                            nc.vector.tensor_tensor(z[:], sview, tslice,
                                                    op=Alu.mult)
                            Ez = sm.tile([128, 512], BF16, tag="Ez",
                                         name="Ez")
                            rs = sm.tile([128, 1], F32, tag="rs", name="rs")
                            nc.scalar.activation(Ez[:], z[:], Act.Exp,
                                                 accum_out=rs[:])
                            ir2 = sm.tile([128, 1], F32, tag="ir2",
                                          name="ir2")
                            nc.vector.reciprocal(ir2[:], rs[:])
                            Ph = sm.tile([128, 512], BF16, tag="Ph",
                                         name="Ph")
                            nc.vector.tensor_scalar(Ph[:], Ez[:], ir2[:],
                                                    None, op0=Alu.mult)
                            if strict and ic == 0:
                                nc.vector.memset(Ph[0:1, :], 0.0)
                            ptr = pT.tile([128, 512], BF16, tag="pT",
                                          name="ptr")
                            for jc in range(4):
                                nc.tensor.transpose(
                                    ptr[:, jc * 128:(jc + 1) * 128],
                                    Ph[:, jc * 128:(jc + 1) * 128],
                                    ident[:])
                            PT = sm.tile([128, 512], BF16, tag="PT",
                                         name="PT", bufs=2)
                            nc.vector.tensor_copy(PT[:], ptr[:])
                            for jc in range(4):
                                nc.tensor.matmul(
                                    pav[:, h * 32:(h + 1) * 32],
                                    PT[:, jc * 128:(jc + 1) * 128],
                                    V[b * 4 + jc][:, h * 32:(h + 1) * 32],
                                    start=(jc == 0), stop=(jc == 3))
                        att_sb = sm.tile([128, 256], BF16, tag="att",
                                         name="att", bufs=2)
                        nc.scalar.activation(att_sb[:], pav[:], Act.Copy)
                        attT = [sm.tile([128, 128], BF16, tag=f"attT{i}",
                                        name="attT", bufs=2)
                                for i in range(2)]
                        for dc in range(2):
                            transpose128(attT[dc][:],
                                         att_sb[:, dc * 128:(dc + 1) * 128])
                        po = pO.tile([128, D], F32, tag="pO", name="po")
                        nc.tensor.matmul(po[:], attT[0][:], Wo[0][:],
                                         start=True, stop=False)
                        nc.tensor.matmul(po[:], attT[1][:], Wo[1][:],
                                         start=False, stop=False)
                        nc.tensor.matmul(po[:], ones1[0:1, 0:128], obr[:],
                                         start=False, stop=True)
                        y = sm.tile([128, D], BF16, tag="ybuf", name="y",
                                    bufs=8)
                        if resid_dram is not None:
                            resid = sm.tile([128, D], F32, tag="xin",
                                            name="resid", bufs=2)
                            nc.sync.dma_start(
                                resid[:],
                                resid_dram[b, ic * 128:(ic + 1) * 128, :])
                            nc.vector.tensor_tensor(y[:], po[:], resid[:],
                                                    op=Alu.add)
                        else:
                            nc.vector.tensor_copy(y[:], po[:])
                        ybufs[pc] = y
                        st6 = sm.tile([128, 6], F32, tag="st6", name="st6")
                        nc.vector.bn_stats(st6[:], y[:])
                        nc.vector.bn_aggr(mvb[:, 2 * pc:2 * pc + 2], st6[:])

            # ---- LN apply (gamma/beta folded) ----
            std = sm.tile([128, PC], F32, tag="std", name="std", bufs=2)
            nc.scalar.activation(std[:], mvb[:, 1:2 * PC:2], Act.Sqrt,
                                 bias=epsT[:], scale=1.0)
            rstd = sm.tile([128, PC], F32, tag="rstd", name="rstd", bufs=2)
            nc.vector.reciprocal(rstd[:], std[:])
            for pc in range(PC):
                nc.vector.tensor_scalar(houts[pc][:], ybufs[pc][:],
                                        mvb[:, 2 * pc:2 * pc + 1],
                                        rstd[:, pc:pc + 1],
                                        op0=Alu.subtract, op1=Alu.mult)
            return houts

        def transpose_chunks(chunks, out_name):
            out = [work.tile([128, BL * S], BF16, tag="xfrm", bufs=6,
                             name=f"{out_name}_{dc}") for dc in range(2)]
            for pc in range(PC):
                for dc in range(2):
                    transpose128(out[dc][:, pc * 128:(pc + 1) * 128],
                                 chunks[pc][:, dc * 128:(dc + 1) * 128])
            return out

        # ================= blocks 1, 2 =================
        xT1 = [xT[(1, 0)], xT[(1, 1)]]
        xT2 = [xT[(2, 0)], xT[(2, 1)]]
        QT1 = projQ(xT1, 'b1_wq', 'b1_qbr', 'QT1')
        V1 = projN(xT1, 'b1_wv', 'b1_vbr', 'V1')
        hq = emit_block('b1', QT1, V1, resid_dram=x1d, out_name='hq')
        hqT = transpose_chunks(hq, 'hqT')
        QT2 = projQ(xT2, 'b2_wq', 'b2_qbr', 'QT2')
        V2 = projN(xT2, 'b2_wv', 'b2_vbr', 'V2')
        ha = emit_block('b2', QT2, V2, resid_dram=x2d, out_name='ha')
        haT = transpose_chunks(ha, 'haT')

        # ================= block 3 =================
        K3T = projT(hqT, 'b3_wk', 'b3_kbT', 'K3T')
        V3 = projN(haT, 'b3_wv', 'b3_vbr', 'V3')
        # c rows -> cTt[grp] [1, 4*1024], col (h%4)*1024 + b*512 + i
        cTt = [rot.tile([1, 4 * 1024], BF16, tag="dbig", bufs=4,
                        name=f"cT_{grp}") for grp in range(2)]
        for pc in range(PC):
            psc = pO.tile([128, H], F32, tag="pO", name="psc")
            for kc in range(2):
                nc.tensor.matmul(psc[:],
                                 K3T[kc][:, pc * 128:(pc + 1) * 128],
                                 C[f"q03__{kc}"][:],
                                 start=(kc == 0), stop=(kc == 1))
            csb = sm.tile([128, H], BF16, tag="csb", name="csb")
            nc.vector.tensor_copy(csb[:], psc[:])
            for grp in range(2):
                ptc = pT.tile([128, 512], BF16, tag="pT", name="ptc")
                for hh in range(4):
                    h = grp * 4 + hh
                    nc.tensor.transpose(ptc[0:1, hh * 128:(hh + 1) * 128],
                                        csb[:, h:h + 1], ident[:])
                src3 = ptc[0:1, 0:512].rearrange("p (h c) -> p h c", h=4)
                dview = cTt[grp].rearrange("p (h c) -> p h c", h=4)[
                    0:1, :, pc * 128:pc * 128 + 128]
                nc.vector.tensor_copy(dview, src3)
        # S3[h] [128, 1024]: broadcast score rows per (h, b)
        S3 = []
        for h in range(H):
            grp, hh = divmod(h, 4)
            ps = p1.tile([128, 1024], F32, tag="p1", name="bc_ps")
            for b in range(BL):
                nc.tensor.matmul(
                    ps[:, b * 512:(b + 1) * 512], ones1[0:1, 0:128],
                    cTt[grp][0:1, hh * 1024 + b * 512:
                             hh * 1024 + (b + 1) * 512],
                    start=True, stop=True)
            Sp = work.tile([128, 1024], BF16, name=f"S3_{h}")
            nc.vector.tensor_copy(Sp[:], ps[:])
            S3.append(Sp)
        h3 = emit_block('b3', None, V3, S3=S3, resid_dram=None,
                        out_name='h3')

        # ================= final stage =================
        # per-head transposed h3: h3Tp[grp] [32, 4*1024], col (h%4)*1024+pc*128
        h3Tp = [rot.tile([32, 4 * 1024], BF16, tag="dbig", bufs=4,
                         name=f"h3Tp_{grp}") for grp in range(2)]
        for pc in range(PC):
            for grp in range(2):
                ptv = pT.tile([128, 512], BF16, tag="pT", name="ptv")
                for hh in range(4):
                    h = grp * 4 + hh
                    nc.tensor.transpose(ptv[0:32, hh * 128:(hh + 1) * 128],
                                        h3[pc][:, h * 32:(h + 1) * 32],
                                        ident[:])
                src3 = ptv[0:32, 0:512].rearrange("p (h c) -> p h c", h=4)
                dview = h3Tp[grp].rearrange("p (h c) -> p h c", h=4)[
                    0:32, :, pc * 128:pc * 128 + 128]
                nc.vector.tensor_copy(dview, src3)
        for pc in range(PC):
            b, ic = divmod(pc, 4)
            vhalves = []
            for half in range(2):
                ps = p1.tile([128, 1024], F32, tag="p1", name="val_ps")
                for hh in range(4):
                    h = half * 4 + hh
                    seg = ps[:, hh * 256:(hh + 1) * 256]
                    nc.tensor.matmul(
                        seg,
                        h3Tp[half][0:32, hh * 1024 + pc * 128:
                                   hh * 1024 + pc * 128 + 128],
                        C[f"lvw__{h}"][:],
                        start=True, stop=False)
                    nc.tensor.matmul(seg, ones1[0:1, 0:128],
                                     C['lvbr'][0:1, h * 256:(h + 1) * 256],
                                     start=False, stop=True)
                val = sm.tile([128, 1024], F32, tag="val", name="val",
                              bufs=2)
                nc.scalar.activation(val[:], ps[:], Act.Sigmoid)
                vhalves.append(val)
            psb = pO.tile([128, D], F32, tag="pO", name="psb")
            for kc in range(2):
                nc.tensor.matmul(psb[:, 0:H],
                                 xT1[kc][:, pc * 128:(pc + 1) * 128],
                                 C[f"keyhT__{kc}"][:],
                                 start=(kc == 0), stop=(kc == 1))
            ea = sm.tile([128, H], F32, tag="ea", name="ea")
            rsa = sm.tile([128, 1], F32, tag="rsa", name="rsa")
            nc.scalar.activation(ea[:], psb[:, 0:H], Act.Exp,
                                 accum_out=rsa[:])
            ira = sm.tile([128, 1], F32, tag="ira", name="ira")
            nc.vector.reciprocal(ira[:], rsa[:])
            alpha = sm.tile([128, H], F32, tag="alpha", name="alpha")
            nc.vector.tensor_scalar(alpha[:], ea[:], ira[:], None,
                                    op0=Alu.mult)
            acc = sm.tile([128, D], F32, tag="acc", name="acc", bufs=2)
            nc.vector.tensor_scalar(acc[:], vhalves[0][:, 0:256],
                                    alpha[:, 0:1], None, op0=Alu.mult)
            for h in range(1, H):
                half, hh = divmod(h, 4)
                acc2 = sm.tile([128, D], F32, tag="acc", name="acc2",
                               bufs=2)
                nc.vector.scalar_tensor_tensor(
                    acc2[:], vhalves[half][:, hh * 256:(hh + 1) * 256],
                    alpha[:, h:h + 1], acc[:],
                    op0=Alu.mult, op1=Alu.add)
                acc = acc2
            nc.sync.dma_start(outd[b, ic * 128:(ic + 1) * 128, :], acc[:])

    nc.compile()
    return nc


_GRAPH_CACHE = {}


def _get_graph(consts, g2):
    key = tuple(np.float32(v) for blk in ('b1', 'b2', 'b3')
                for v in g2[blk])
    if key not in _GRAPH_CACHE:
        _GRAPH_CACHE[key] = _build(consts, g2)
    return _GRAPH_CACHE[key]


def kernel(**inputs):
    consts, g2 = _host_prep(inputs)
    nc = _get_graph(consts, g2)
    q = np.ascontiguousarray(np.asarray(inputs['q_emb'], np.float32))
    qa = np.ascontiguousarray(np.asarray(inputs['qa_emb'], np.float32))
    in_maps = []
    for core in range(NCORES):
        m = {'x1': q[core * BL:(core + 1) * BL],
             'x2': qa[core * BL:(core + 1) * BL]}
        m.update(consts)
        in_maps.append(m)
    res = run_bass_kernel_spmd(nc, in_maps, core_ids=list(range(NCORES)))
    out = np.concatenate([res.results[c]['out'] for c in range(NCORES)],
                         axis=0)
    return out.astype(np.float32)


# revision 20
# speedup vs baseline: 103.8160x; 103.8160x over previous
"""Trainium2 Bass kernel for nn_Architecture_11879879540882 (AKT-style
monotonic sparse attention), data-parallel over batch on 8 NeuronCores.

Self-contained: hardcodes shapes B=16,S=512,D=256,H=8,DK=32, shards the batch
2-per-core, runs one Bass graph SPMD via run_bass_kernel_spmd, gathers output.

Algorithm notes (validated vs the jax reference in numpy, rel err ~1.4e-3 in
full bf16):
 - blocks 1/2: k-projection shares q weights and inputs -> K == Q.
 - masked softmax + cumsum distance statistics collapse into ONE reversed
   masked scan: state=(E+state)*mask -> all suffix sums + masked row total.
 - dist = sqrt(suffix*pos*g^2/r), total = exp(-dist)   [g = -softplus(gamma)]
 - second softmax is UNMASKED (reproduces the reference's non-inplace
   masked_fill bug); the 1e-5 clip on `total` is skipped (~1e-4 error).
 - all LayerNorm gamma/beta fold into downstream weights on host.
 - block3's query is position-independent -> its score rows are broadcasts.

Hardware constraints honored: matmul partition bases must be in {0,32,64}
(per-head tensors packed along the free axis); SBUF budget ~192KB/partition
(half-block phase granularity + aggressive tile-tag slot reuse).
"""
import sys
import numpy as np

for _p in ('/opt/trn_rl_repo',):
    if _p not in sys.path:
        sys.path.append(_p)

import ml_dtypes
import concourse.bass as bass
import concourse.bacc as bacc
import concourse.tile as tile
import concourse.mybir as mybir
from concourse.bass_utils import run_bass_kernel_spmd

F32 = mybir.dt.float32
BF16 = mybir.dt.bfloat16
Alu = mybir.AluOpType
Act = mybir.ActivationFunctionType
NPBF = ml_dtypes.bfloat16

B, S, D, H, DK = 16, 512, 256, 8, 32
NCORES = 8
BL = B // NCORES          # local batches per core = 2
PC = BL * 4               # 128-row position chunks per core = 8
LN_EPS = 1e-5

REV = (slice(None), slice(None, None, -1))


def _softplus(x):
    return np.logaddexp(0.0, x)


def _host_prep(inp):
    """Parameter preprocessing on host. Returns (consts dict, g2 dict)."""
    p = {k: np.asarray(v, np.float32) for k, v in inp.items()}
    c = {}
    s4 = np.float32(DK ** -0.25)
    bf = lambda x: np.ascontiguousarray(np.asarray(x, np.float32)).astype(NPBF)
    colpack = lambda b: np.ascontiguousarray(
        np.asarray(b, np.float32).reshape(2, 128).T).astype(np.float32)

    for blk in ('b1', 'b2'):
        c[blk + '_wq'] = bf(p[blk + '_qw'] * s4)
        c[blk + '_qbr'] = bf((p[blk + '_qb'] * s4)[None, :])
        c[blk + '_wv'] = bf(p[blk + '_vw'])
        c[blk + '_vbr'] = bf(p[blk + '_vb'][None, :])
        c[blk + '_wo'] = bf(p[blk + '_ow'])
        c[blk + '_obr'] = bf(p[blk + '_ob'][None, :])
    know = p['know'][0, 0]
    q03 = ((know @ p['b3_qw'] + p['b3_qb']) / np.sqrt(DK)).reshape(H, DK)
    Q03 = np.zeros((D, H), np.float32)
    for h in range(H):
        Q03[h * DK:(h + 1) * DK, h] = q03[h]
    c['q03'] = bf(Q03)
    g1, be1 = p['b1_lng'], p['b1_lnb']
    c['b3_wk'] = bf(p['b3_kw'] * g1[:, None])
    c['b3_kbT'] = colpack(p['b3_kb'] + be1 @ p['b3_kw'])
    g2_, be2 = p['b2_lng'], p['b2_lnb']
    c['b3_wv'] = bf(p['b3_vw'] * g2_[:, None])
    c['b3_vbr'] = bf((p['b3_vb'] + be2 @ p['b3_vw'])[None, :])
    c['b3_wo'] = bf(p['b3_ow'])
    c['b3_obr'] = bf((p['b3_ob'] + know)[None, :])
    g3, be3 = p['b3_lng'], p['b3_lnb']
    lvw = np.zeros((H, DK, D), np.float32)
    lvb = np.zeros((H, D), np.float32)
    for h in range(H):
        sl = slice(h * DK, (h + 1) * DK)
        lvw[h] = p['lv_w'] * g3[sl][:, None]
        lvb[h] = p['lv_b'] + be3[sl] @ p['lv_w']
    c['lvw'] = bf(lvw)                            # -> lvw__h [32,256]
    c['lvbr'] = bf(lvb.reshape(1, H * D))         # [1, 2048]
    know_r = know.reshape(H, DK)
    keyh = 1.0 / (1.0 + np.exp(-(know_r @ p['lk_w'] + p['lk_b'])))
    c['keyhT'] = bf(keyh.T)                       # [D, H]

    # padded inclusive mask: mpad[ic][p, j] = (j <= i_p), j in [0, 512];
    # strict mask is the shifted view mpad[:, 1:513].
    i = np.arange(S + 1, dtype=np.int64)
    mpad = np.zeros((4, 128, S + 1), np.float32)
    pos = np.zeros((4, 128, S), np.float32)
    for ic in range(4):
        ii = np.arange(ic * 128, (ic + 1) * 128, dtype=np.int64)[:, None]
        mpad[ic] = (i[None, :] <= ii)
        pos[ic] = np.abs(ii - i[None, :S])
    for blk in ('b1', 'b2', 'b3'):
        g2v = (_softplus(p[blk + '_gam'][:, 0, 0]) ** 2).astype(np.float32)
        c['g2b_' + blk] = np.ascontiguousarray(
            np.broadcast_to(g2v[None, :], (128, H))).astype(np.float32)
    c['mpad'] = bf(mpad)
    c['posm'] = bf(pos)
    c['ident'] = bf(np.eye(128))

    flat = {}
    for name, a in c.items():
        if a.ndim == 2 and a.shape[0] > 128:
            for kc in range(a.shape[0] // 128):
                flat[f"{name}__{kc}"] = np.ascontiguousarray(
                    a[kc * 128:(kc + 1) * 128])
        elif a.ndim == 3:
            for kc in range(a.shape[0]):
                flat[f"{name}__{kc}"] = np.ascontiguousarray(a[kc])
        else:
            flat[name] = a
    g2 = {blk: [float(v) for v in
                (_softplus(p[blk + '_gam'][:, 0, 0]) ** 2)]
          for blk in ('b1', 'b2', 'b3')}
    return flat, g2


_NPDT = {np.dtype(np.float32): F32, np.dtype(NPBF): BF16}


def _build(consts, g2):
    """Builds the per-core Bass graph (BL local batches)."""
    nc = bacc.Bacc("TRN2", target_bir_lowering=False, debug=False)

    x1d = nc.dram_tensor("x1", [BL, S, D], F32, kind="ExternalInput")
    x2d = nc.dram_tensor("x2", [BL, S, D], F32, kind="ExternalInput")
    outd = nc.dram_tensor("out", [BL, S, D], F32, kind="ExternalOutput")
    cd = {name: nc.dram_tensor(name, list(a.shape), _NPDT[a.dtype],
                               kind="ExternalInput")
          for name, a in consts.items()}

    from contextlib import ExitStack
    with tile.TileContext(nc) as tc, ExitStack() as _ps:
        sb = _ps.enter_context(tc.tile_pool(name="const", bufs=1))
        work = _ps.enter_context(tc.tile_pool(name="work", bufs=1))
        rot = _ps.enter_context(tc.tile_pool(name="rot", bufs=4))
        sm = _ps.enter_context(tc.tile_pool(name="sm", bufs=3))
        p1 = _ps.enter_context(tc.tile_pool(name="p1", bufs=2, space="PSUM"))
        pT = _ps.enter_context(tc.tile_pool(name="pT", bufs=2, space="PSUM"))
        pAV = _ps.enter_context(tc.tile_pool(name="pAV", bufs=1, space="PSUM"))
        pO = _ps.enter_context(tc.tile_pool(name="pO", bufs=1, space="PSUM"))

        # ---------- constants ----------
        # DMA order = need order: block1's weights and the masks gate the
        # first compute; block2/3/final constants can land later.
        def _prio(name):
            for i, k in enumerate(('ident', 'b1_', 'mpad', 'posm', 'g2b_b1',
                                   'b2_', 'g2b_b2', 'b3_', 'q03', 'g2b_b3',
                                   'lv', 'key')):
                if name.startswith(k):
                    return i
            return 99
        C = {}
        for name in sorted(cd, key=_prio):
            ap = cd[name]
            t = sb.tile(list(ap.shape), ap.dtype, name="c_" + name)
            nc.sync.dma_start(t[:], ap[:])
            C[name] = t
        ones1 = sb.tile([1, 512], BF16, name="ones1")
        nc.vector.memset(ones1[:], 1.0)
        epsT = sb.tile([128, 1], F32, name="epsT")
        nc.vector.memset(epsT[:], LN_EPS)
        ident = C['ident']

        def transpose128(dst, src):
            """dst[128,128] SBUF bf16 = src.T via PE + DVE copy."""
            pt = pT.tile([128, 512], BF16, tag="pT", name="pt_t")
            nc.tensor.transpose(pt[:, 0:128], src, ident[:])
            nc.vector.tensor_copy(dst, pt[:, 0:128])

        # ---------- input prep: transposed bf16 copies of x1/x2 ----------
        xT = {}
        for xi, xd in ((1, x1d), (2, x2d)):
            for dc in range(2):
                xT[(xi, dc)] = work.tile([128, BL * S], BF16, tag="xfrm",
                                         bufs=6, name=f"xT{xi}_{dc}")
            for pc in range(PC):
                b, ic = divmod(pc, 4)
                t = sm.tile([128, D], F32, tag="xin", name="xin", bufs=2)
                nc.sync.dma_start(t[:], xd[b, ic * 128:(ic + 1) * 128, :])
                tb = sm.tile([128, D], BF16, tag="xbf", name="xbf_t", bufs=2)
                nc.vector.tensor_copy(tb[:], t[:])
                for dc in range(2):
                    transpose128(xT[(xi, dc)][:, pc * 128:(pc + 1) * 128],
                                 tb[:, dc * 128:(dc + 1) * 128])

        # ---------- projections ----------
        def projQ(xTloc, wname, brname, out_name):
            """Head-packed transposed projection QTp [32, H*BL*S]:
            head h occupies cols [h*1024, (h+1)*1024)."""
            QTp = work.tile([32, H * BL * S], BF16, name=out_name,
                            tag="QTp", bufs=1)
            qbr = C[brname]
            for h in range(H):
                for half in range(BL):
                    ps = p1.tile([32, 512], F32, tag="p1", name="projQ_ps")
                    for kc in range(2):
                        nc.tensor.matmul(
                            ps[:],
                            C[f"{wname}__{kc}"][:, h * 32:(h + 1) * 32],
                            xTloc[kc][:, half * 512:(half + 1) * 512],
                            start=(kc == 0), stop=False)
                    nc.tensor.matmul(ps[:], qbr[0:1, h * 32:(h + 1) * 32],
                                     ones1[0:1, :], start=False, stop=True)
                    nc.scalar.activation(
                        QTp[:, h * 1024 + half * 512:
                            h * 1024 + (half + 1) * 512], ps[:], Act.Copy)
            return QTp

        def projT(xTloc, wname, bTname, out_name):
            """Chunk-transposed projection out[dc][128, BL*S] (for K3T)."""
            out = [work.tile([128, BL * S], BF16, tag="xfrm", bufs=6,
                             name=f"{out_name}_{dc}") for dc in range(2)]
            bT = C[bTname]
            for dc in range(2):
                for hh in range(BL):
                    ps = p1.tile([128, 1024], F32, tag="p1", name="projT_ps")
                    for kc in range(2):
                        nc.tensor.matmul(
                            ps[:, 0:512],
                            C[f"{wname}__{kc}"][:, dc * 128:(dc + 1) * 128],
                            xTloc[kc][:, hh * 512:(hh + 1) * 512],
                            start=(kc == 0), stop=(kc == 1))
                    nc.scalar.activation(out[dc][:, hh * 512:(hh + 1) * 512],
                                         ps[:, 0:512], Act.Identity,
                                         bias=bT[:, dc:dc + 1], scale=1.0)
            return out

        def projN(xTloc, wname, brname, out_name):
            """Natural projection, head-packed with a ones column:
            out[pc] [128, 8*33]: head h = cols [33h, 33h+32), col 33h+32 = 1.
            The ones column makes the AV matmul also emit softmax-2 row
            sums."""
            out = [work.tile([128, H * 33], BF16, tag="Vt", bufs=8,
                             name=f"{out_name}_{pc}") for pc in range(PC)]
            br = C[brname]
            for pc in range(PC):
                ps = pO.tile([128, D], F32, tag="pO", name="projN_ps")
                for kc in range(2):
                    nc.tensor.matmul(ps[:],
                                     xTloc[kc][:, pc * 128:(pc + 1) * 128],
                                     C[f"{wname}__{kc}"],
                                     start=(kc == 0), stop=False)
                nc.tensor.matmul(ps[:], ones1[0:1, 0:128], br[:],
                                 start=False, stop=True)
                ov = out[pc].rearrange("p (h c) -> p h c", c=33)
                nc.scalar.activation(ov[:, :, 0:32],
                                     ps.rearrange("p (h c) -> p h c", c=32),
                                     Act.Copy)
                nc.gpsimd.memset(ov[:, :, 32:33], 1.0)
            return out

        # ---------- one attention block (half-block phase granularity) ----
        def emit_block(blk, QTp, V, S3=None, resid_dram=None,
                       out_name="hout"):
            g2l = g2[blk]
            g2b = C['g2b_' + blk]
            strict = (blk == 'b3')
            mpad = [C[f"mpad__{ic}"] for ic in range(4)]
            masks = [mpad[ic][:, 1:513] if strict else mpad[ic][:, 0:512]
                     for ic in range(4)]
            posm = [C[f"posm__{ic}"] for ic in range(4)]

            def qk_matmul(ps_out, b, h, ic):
                base = h * 1024 + b * 512
                nc.tensor.matmul(
                    ps_out,
                    QTp[:, base + ic * 128: base + ic * 128 + 128],
                    QTp[:, base: base + 512],
                    start=True, stop=True)

            houts = [work.tile([128, D], BF16, tag="hblk", bufs=8,
                               name=f"{out_name}_{pc}") for pc in range(PC)]
            ybufs = {}
            mvb = sm.tile([128, 2 * PC], F32, tag="mvb", name="mvb", bufs=2)
            Wo = [C[blk + '_wo__0'], C[blk + '_wo__1']]
            obr = C[blk + '_obr']

            for hblk in range(2):           # half-block: ics {0,1} / {2,3}
                ics = (2 * hblk, 2 * hblk + 1)
                bigA = {}
                for b in range(BL):
                    for ic in ics:
                        bigA[b * 4 + ic] = work.tile(
                            [128, 8 * 512], BF16, tag="bigA", bufs=4,
                            name="bigA_t")

                # ---- phase 1 ----
                def do_scan(Rv, Eview, ic):
                    nc.gpsimd.memset(Rv[:, 512:513], 0.0)
                    nc.vector.tensor_tensor_scan(
                        Rv[:, 0:512][REV], Eview[REV], masks[ic][REV],
                        0.0, op0=Alu.add, op1=Alu.mult)

                if not strict:
                    for ic in ics:
                        Rvs = {}
                        r8 = [sm.tile([128, H], F32, tag="r8", name="r8",
                                      bufs=2) for _ in range(BL)]
                        for h in range(H):
                            ps = p1.tile([128, 1024], F32, tag="p1",
                                         name="qk_ps")
                            for b in range(BL):
                                qk_matmul(ps[:, b * 512:(b + 1) * 512],
                                          b, h, ic)
                            Epair = rot.tile([128, 1024], BF16, tag="Epair",
                                             name="Epair")
                            nc.scalar.activation(Epair[:], ps[:], Act.Exp)
                            for b in range(BL):
                                Rv = sm.tile([128, 513], BF16, tag="Rv",
                                             name="Rv", bufs=18)
                                do_scan(Rv, Epair[:, b * 512:(b + 1) * 512],
                                        ic)
                                nc.vector.tensor_copy(r8[b][:, h:h + 1],
                                                      Rv[:, 0:1])
                                Rvs[(b, h)] = Rv
                        for b in range(BL):
                            r8m = sm.tile([128, H], F32, tag="r8m",
                                          name="r8m", bufs=2)
                            nc.vector.tensor_scalar(r8m[:], r8[b][:], 1e-30,
                                                    None, op0=Alu.max)
                            rc = sm.tile([128, H], F32, tag="rc", name="rc",
                                         bufs=2)
                            nc.vector.reciprocal(rc[:], r8m[:])
                            rgb = sm.tile([128, H], F32, tag="rgb",
                                          name="rgb", bufs=2)
                            nc.vector.tensor_tensor(rgb[:], rc[:], g2b[:],
                                                    op=Alu.mult)
                            for h in range(H):
                                nc.vector.scalar_tensor_tensor(
                                    bigA[b * 4 + ic][:, h * 512:(h + 1) * 512],
                                    Rvs[(b, h)][:, 1:513], rgb[:, h:h + 1],
                                    posm[ic][:], op0=Alu.mult, op1=Alu.mult)
                else:
                    for h in range(H):
                        for b in range(BL):
                            Eb3 = rot.tile([128, 512], BF16, tag="Eb3",
                                           name="Eb3")
                            nc.scalar.activation(
                                Eb3[:], S3[h][:, b * 512:(b + 1) * 512],
                                Act.Exp)
                            for ic in ics:
                                Rv = sm.tile([128, 513], BF16, tag="Rv",
                                             name="Rv", bufs=18)
                                do_scan(Rv, Eb3[:], ic)
                                r1 = sm.tile([128, 1], F32, tag="r1",
                                             name="r1")
                                nc.vector.tensor_scalar(r1[:], Rv[:, 0:1],
                                                        1e-30, None,
                                                        op0=Alu.max)
                                ir = sm.tile([128, 1], F32, tag="ir",
                                             name="ir")
                                nc.vector.reciprocal(ir[:], r1[:])
                                rg = sm.tile([128, 1], F32, tag="rg",
                                             name="rg")
                                nc.vector.tensor_scalar(rg[:], ir[:],
                                                        float(g2l[h]),
                                                        None, op0=Alu.mult)
                                nc.vector.scalar_tensor_tensor(
                                    bigA[b * 4 + ic][:, h * 512:(h + 1) * 512],
                                    Rv[:, 1:513], rg[:], posm[ic][:],
                                    op0=Alu.mult, op1=Alu.mult)

                # ---- phase 2: total = exp(-sqrt(v)) ----
                # (all Sqrt first, then all Exp: each Sqrt<->Exp transition
                # costs a ~2.7us ACT table-set reload)
                dbigs = {}
                for g in sorted(bigA):
                    dbig = rot.tile([128, 8 * 512], BF16, tag="dbig",
                                    name="dbig", bufs=4)
                    nc.scalar.activation(dbig[:], bigA[g][:], Act.Sqrt)
                    dbigs[g] = dbig
                for g in sorted(bigA):
                    nc.scalar.activation(bigA[g][:], dbigs[g][:], Act.Exp,
                                         scale=-1.0)

                # ---- phase 3 ----
                for b in range(BL):
                    for ic in ics:
                        g = b * 4 + ic
                        pc = g
                        pav = pAV.tile([128, H * 33], F32, tag="pAV",
                                       name="pav")
                        for hp in range(4):
                            zpair = sm.tile([128, 1024], BF16, tag="z",
                                            name="z", bufs=3)
                            for ho in range(2):
                                h = 2 * hp + ho
                                tslice = bigA[g][:, h * 512:(h + 1) * 512]
                                if not strict:
                                    ps3 = p1.tile([128, 1024], F32,
                                                  tag="p1", name="s3_ps")
                                    qk_matmul(ps3[:, 0:512], b, h, ic)
                                    sview = ps3[:, 0:512]
                                else:
                                    sview = S3[h][:, b * 512:(b + 1) * 512]
                                nc.vector.tensor_tensor(
                                    zpair[:, ho * 512:(ho + 1) * 512],
                                    sview, tslice, op=Alu.mult)
                            Ezp = sm.tile([128, 1024], BF16, tag="Ez",
                                          name="Ez", bufs=3)
                            nc.scalar.activation(Ezp[:], zpair[:], Act.Exp)
                            if strict and ic == 0:
                                nc.gpsimd.memset(Ezp[0:1, :], 0.0)
                            for ho in range(2):
                                h = 2 * hp + ho
                                Ev = Ezp[:, ho * 512:(ho + 1) * 512]
                                ptr = pT.tile([128, 512], BF16, tag="pT",
                                              name="ptr")
                                for jc in range(4):
                                    nc.tensor.transpose(
                                        ptr[:, jc * 128:(jc + 1) * 128],
                                        Ev[:, jc * 128:(jc + 1) * 128],
                                        ident[:])
                                PT = sm.tile([128, 512], BF16, tag="PT",
                                             name="PT", bufs=2)
                                if ho == 0:
                                    nc.vector.tensor_copy(PT[:], ptr[:])
                                else:
                                    nc.scalar.activation(PT[:], ptr[:],
                                                         Act.Copy)
                                for jc in range(4):
                                    nc.tensor.matmul(
                                        pav[:, h * 33:(h + 1) * 33],
                                        PT[:, jc * 128:(jc + 1) * 128],
                                        V[b * 4 + jc][:, h * 33:(h + 1) * 33],
                                        start=(jc == 0), stop=(jc == 3))
                        pav3 = pav.rearrange("p (h c) -> p h c", c=33)
                        rsm = sm.tile([128, H], F32, tag="rsm", name="rsm")
                        nc.vector.tensor_scalar(rsm[:], pav3[:, :, 32:33],
                                                1e-30, None, op0=Alu.max)
                        rsi = sm.tile([128, H], F32, tag="rsi", name="rsi")
                        nc.vector.reciprocal(rsi[:], rsm[:])
                        att_sb = sm.tile([128, 256], BF16, tag="att",
                                         name="att", bufs=2)
                        nc.scalar.activation(
                            att_sb.rearrange("p (h c) -> p h c", c=32),
                            pav3[:, :, 0:32], Act.Copy)
                        att_n = sm.tile([128, 256], BF16, tag="attn",
                                        name="attn", bufs=2)
                        for h in range(H):
                            nc.vector.tensor_scalar(
                                att_n[:, h * 32:(h + 1) * 32],
                                att_sb[:, h * 32:(h + 1) * 32],
                                rsi[:, h:h + 1], None, op0=Alu.mult)
                        attT = [sm.tile([128, 128], BF16, tag=f"attT{i}",
                                        name="attT", bufs=2)
                                for i in range(2)]
                        for dc in range(2):
                            transpose128(attT[dc][:],
                                         att_n[:, dc * 128:(dc + 1) * 128])
                        po = pO.tile([128, D], F32, tag="pO", name="po")
                        nc.tensor.matmul(po[:], attT[0][:], Wo[0][:],
                                         start=True, stop=False)
                        nc.tensor.matmul(po[:], attT[1][:], Wo[1][:],
                                         start=False, stop=False)
                        nc.tensor.matmul(po[:], ones1[0:1, 0:128], obr[:],
                                         start=False, stop=True)
                        y = sm.tile([128, D], BF16, tag="ybuf", name="y",
                                    bufs=8)
                        if resid_dram is not None:
                            resid = sm.tile([128, D], F32, tag="xin",
                                            name="resid", bufs=2)
                            nc.sync.dma_start(
                                resid[:],
                                resid_dram[b, ic * 128:(ic + 1) * 128, :])
                            nc.vector.tensor_tensor(y[:], po[:], resid[:],
                                                    op=Alu.add)
                        else:
                            nc.vector.tensor_copy(y[:], po[:])
                        ybufs[pc] = y
                        st6 = sm.tile([128, 6], F32, tag="st6", name="st6")
                        nc.vector.bn_stats(st6[:], y[:])
                        nc.vector.bn_aggr(mvb[:, 2 * pc:2 * pc + 2], st6[:])

            # ---- LN apply (gamma/beta folded) ----
            std = sm.tile([128, PC], F32, tag="std", name="std", bufs=2)
            nc.scalar.activation(std[:], mvb[:, 1:2 * PC:2], Act.Sqrt,
                                 bias=epsT[:], scale=1.0)
            rstd = sm.tile([128, PC], F32, tag="rstd", name="rstd", bufs=2)
            nc.vector.reciprocal(rstd[:], std[:])
            for pc in range(PC):
                nc.vector.tensor_scalar(houts[pc][:], ybufs[pc][:],
                                        mvb[:, 2 * pc:2 * pc + 1],
                                        rstd[:, pc:pc + 1],
                                        op0=Alu.subtract, op1=Alu.mult)
            return houts

        def transpose_chunks(chunks, out_name):
            out = [work.tile([128, BL * S], BF16, tag="xfrm", bufs=6,
                             name=f"{out_name}_{dc}") for dc in range(2)]
            for pc in range(PC):
                for dc in range(2):
                    transpose128(out[dc][:, pc * 128:(pc + 1) * 128],
                                 chunks[pc][:, dc * 128:(dc + 1) * 128])
            return out

        # ================= blocks 1, 2 =================
        xT1 = [xT[(1, 0)], xT[(1, 1)]]
        xT2 = [xT[(2, 0)], xT[(2, 1)]]
        QT1 = projQ(xT1, 'b1_wq', 'b1_qbr', 'QT1')
        V1 = projN(xT1, 'b1_wv', 'b1_vbr', 'V1')
        hq = emit_block('b1', QT1, V1, resid_dram=x1d, out_name='hq')
        hqT = transpose_chunks(hq, 'hqT')
        QT2 = projQ(xT2, 'b2_wq', 'b2_qbr', 'QT2')
        V2 = projN(xT2, 'b2_wv', 'b2_vbr', 'V2')
        ha = emit_block('b2', QT2, V2, resid_dram=x2d, out_name='ha')
        haT = transpose_chunks(ha, 'haT')

        # ================= block 3 =================
        K3T = projT(hqT, 'b3_wk', 'b3_kbT', 'K3T')
        V3 = projN(haT, 'b3_wv', 'b3_vbr', 'V3')
        # c rows -> cTt[grp] [1, 4*1024], col (h%4)*1024 + b*512 + i
        cTt = [rot.tile([1, 4 * 1024], BF16, tag="dbig", bufs=4,
                        name=f"cT_{grp}") for grp in range(2)]
        for pc in range(PC):
            psc = pO.tile([128, H], F32, tag="pO", name="psc")
            for kc in range(2):
                nc.tensor.matmul(psc[:],
                                 K3T[kc][:, pc * 128:(pc + 1) * 128],
                                 C[f"q03__{kc}"][:],
                                 start=(kc == 0), stop=(kc == 1))
            csb = sm.tile([128, H], BF16, tag="csb", name="csb")
            nc.vector.tensor_copy(csb[:], psc[:])
            for grp in range(2):
                ptc = pT.tile([128, 512], BF16, tag="pT", name="ptc")
                for hh in range(4):
                    h = grp * 4 + hh
                    nc.tensor.transpose(ptc[0:1, hh * 128:(hh + 1) * 128],
                                        csb[:, h:h + 1], ident[:])
                src3 = ptc[0:1, 0:512].rearrange("p (h c) -> p h c", h=4)
                dview = cTt[grp].rearrange("p (h c) -> p h c", h=4)[
                    0:1, :, pc * 128:pc * 128 + 128]
                nc.vector.tensor_copy(dview, src3)
        # S3[h] [128, 1024]: broadcast score rows per (h, b)
        S3 = []
        for h in range(H):
            grp, hh = divmod(h, 4)
            ps = p1.tile([128, 1024], F32, tag="p1", name="bc_ps")
            for b in range(BL):
                nc.tensor.matmul(
                    ps[:, b * 512:(b + 1) * 512], ones1[0:1, 0:128],
                    cTt[grp][0:1, hh * 1024 + b * 512:
                             hh * 1024 + (b + 1) * 512],
                    start=True, stop=True)
            Sp = work.tile([128, 1024], BF16, name=f"S3_{h}")
            nc.vector.tensor_copy(Sp[:], ps[:])
            S3.append(Sp)
        h3 = emit_block('b3', None, V3, S3=S3, resid_dram=None,
                        out_name='h3')

        # ================= final stage =================
        # per-head transposed h3: h3Tp[grp] [32, 4*1024], col (h%4)*1024+pc*128
        h3Tp = [rot.tile([32, 4 * 1024], BF16, tag="dbig", bufs=4,
                         name=f"h3Tp_{grp}") for grp in range(2)]
        for pc in range(PC):
            for grp in range(2):
                ptv = pT.tile([128, 512], BF16, tag="pT", name="ptv")
                for hh in range(4):
                    h = grp * 4 + hh
                    nc.tensor.transpose(ptv[0:32, hh * 128:(hh + 1) * 128],
                                        h3[pc][:, h * 32:(h + 1) * 32],
                                        ident[:])
                src3 = ptv[0:32, 0:512].rearrange("p (h c) -> p h c", h=4)
                dview = h3Tp[grp].rearrange("p (h c) -> p h c", h=4)[
                    0:32, :, pc * 128:pc * 128 + 128]
                nc.vector.tensor_copy(dview, src3)
        for pc in range(PC):
            b, ic = divmod(pc, 4)
            vhalves = []
            for half in range(2):
                ps = p1.tile([128, 1024], F32, tag="p1", name="val_ps")
                for hh in range(4):
                    h = half * 4 + hh
                    seg = ps[:, hh * 256:(hh + 1) * 256]
                    nc.tensor.matmul(
                        seg,
                        h3Tp[half][0:32, hh * 1024 + pc * 128:
                                   hh * 1024 + pc * 128 + 128],
                        C[f"lvw__{h}"][:],
                        start=True, stop=False)
                    nc.tensor.matmul(seg, ones1[0:1, 0:128],
                                     C['lvbr'][0:1, h * 256:(h + 1) * 256],
                                     start=False, stop=True)
                val = sm.tile([128, 1024], F32, tag="val", name="val",
                              bufs=2)
                nc.scalar.activation(val[:], ps[:], Act.Sigmoid)
                vhalves.append(val)
            psb = pO.tile([128, D], F32, tag="pO", name="psb")
            for kc in range(2):
                nc.tensor.matmul(psb[:, 0:H],
                                 xT1[kc][:, pc * 128:(pc + 1) * 128],
                                 C[f"keyhT__{kc}"][:],
                                 start=(kc == 0), stop=(kc == 1))
            ea = sm.tile([128, H], F32, tag="ea", name="ea")
            rsa = sm.tile([128, 1], F32, tag="rsa", name="rsa")
            nc.scalar.activation(ea[:], psb[:, 0:H], Act.Exp,
                                 accum_out=rsa[:])
            ira = sm.tile([128, 1], F32, tag="ira", name="ira")
            nc.vector.reciprocal(ira[:], rsa[:])
            alpha = sm.tile([128, H], F32, tag="alpha", name="alpha")
            nc.vector.tensor_scalar(alpha[:], ea[:], ira[:], None,
                                    op0=Alu.mult)
            acc = sm.tile([128, D], F32, tag="acc", name="acc", bufs=2)
            nc.vector.tensor_scalar(acc[:], vhalves[0][:, 0:256],
                                    alpha[:, 0:1], None, op0=Alu.mult)
            for h in range(1, H):
                half, hh = divmod(h, 4)
                acc2 = sm.tile([128, D], F32, tag="acc", name="acc2",
                               bufs=2)
                nc.vector.scalar_tensor_tensor(
                    acc2[:], vhalves[half][:, hh * 256:(hh + 1) * 256],
                    alpha[:, h:h + 1], acc[:],
                    op0=Alu.mult, op1=Alu.add)
                acc = acc2
            nc.sync.dma_start(outd[b, ic * 128:(ic + 1) * 128, :], acc[:])

    nc.compile()
    return nc


_GRAPH_CACHE = {}


def _get_graph(consts, g2):
    key = tuple(np.float32(v) for blk in ('b1', 'b2', 'b3')
                for v in g2[blk])
    if key not in _GRAPH_CACHE:
        _GRAPH_CACHE[key] = _build(consts, g2)
    return _GRAPH_CACHE[key]


def kernel(**inputs):
    consts, g2 = _host_prep(inputs)
    nc = _get_graph(consts, g2)
    q = np.ascontiguousarray(np.asarray(inputs['q_emb'], np.float32))
    qa = np.ascontiguousarray(np.asarray(inputs['qa_emb'], np.float32))
    in_maps = []
    for core in range(NCORES):
        m = {'x1': q[core * BL:(core + 1) * BL],
             'x2': qa[core * BL:(core + 1) * BL]}
        m.update(consts)
        in_maps.append(m)
    res = run_bass_kernel_spmd(nc, in_maps, core_ids=list(range(NCORES)))
    out = np.concatenate([res.results[c]['out'] for c in range(NCORES)],
                         axis=0)
    return out.astype(np.float32)
